# revision 7
# baseline (speedup 1.0000x reference)
"""GraphVAE forward pass on 8 Trainium2 NeuronCores (Bass/Tile).

Row-shards the N=8192 graph across 8 cores. Each core holds a [N, N/8]
transposed bf16 slice of adj (columns = its rows of adj) and streams it three
times: degree reduction, conv1 aggregation, conv2 aggregation. Activations
cross cores via on-device AllGather (dinv, scaled hidden h, latent z).
The z @ z^T decode keeps its [N/8, N] output rows local.

Self-contained: takes full inputs, returns full outputs (adj_recon, mu, logvar).
"""
import numpy as np

import concourse.bacc as bacc
import concourse.mybir as mybir
from concourse.tile import TileContext
from concourse.masks import make_identity
from concourse import bass_utils

F, H, Z = 64, 128, 32
NCORES = 8
P = 128
F32 = mybir.dt.float32
BF16 = mybir.dt.bfloat16
RG = [list(range(NCORES))]

_CACHE = {}


def build_bass(N=8192, stream_bufs=6):
    R = N // NCORES          # rows per core
    KT = N // P              # contraction tiles
    RT = R // P              # row tiles per core
    NCH = N // 512           # decode column chunks

    nc = bacc.Bacc(
        "TRN2", target_bir_lowering=False, debug=False, num_devices=NCORES
    )

    # ---- per-core I/O ----
    aT = nc.dram_tensor("aT", [N, R], BF16, kind="ExternalInput")
    xp = nc.dram_tensor("xp", [P, KT * F], F32, kind="ExternalInput")
    xop = nc.dram_tensor("xop", [P, RT * F], F32, kind="ExternalInput")
    epsT = nc.dram_tensor("epsT", [Z, R], F32, kind="ExternalInput")
    W1 = nc.dram_tensor("W1", [F, H], F32, kind="ExternalInput")
    b1 = nc.dram_tensor("b1", [H, 1], F32, kind="ExternalInput")
    Wmu = nc.dram_tensor("Wmu", [H, Z], F32, kind="ExternalInput")
    bmu = nc.dram_tensor("bmu", [Z, 1], F32, kind="ExternalInput")
    Wlv = nc.dram_tensor("Wlv", [H, Z], F32, kind="ExternalInput")
    blv = nc.dram_tensor("blv", [Z, 1], F32, kind="ExternalInput")

    rec = nc.dram_tensor("rec", [R, N], F32, kind="ExternalOutput")
    muT_o = nc.dram_tensor("muT", [Z, R], F32, kind="ExternalOutput")
    lvT_o = nc.dram_tensor("lvT", [Z, R], F32, kind="ExternalOutput")

    with TileContext(nc) as tc:
        with (
            tc.tile_pool(name="const", bufs=1) as cpool,
            tc.tile_pool(name="dram", bufs=1, space="DRAM") as dram,
            tc.tile_pool(name="persist", bufs=1) as ppool,
        ):
            # ---- constants ----
            identb = cpool.tile([P, P], BF16, tag="identb")
            make_identity(nc, identb[:])
            identf = cpool.tile([P, P], F32, tag="identf")
            make_identity(nc, identf[:])
            ones_b = cpool.tile([P, 1], BF16, tag="ones_b")
            nc.vector.memset(ones_b[:], 1.0)
            ones_row = cpool.tile([1, P], F32, tag="ones_row")
            nc.vector.memset(ones_row[:], 1.0)
            W1s = cpool.tile([F, H], F32, tag="W1s")
            nc.sync.dma_start(W1s[:], W1[:])
            b1s = cpool.tile([H, 1], F32, tag="b1s")
            nc.sync.dma_start(b1s[:], b1[:])
            Wmus = cpool.tile([H, Z], F32, tag="Wmus")
            nc.sync.dma_start(Wmus[:], Wmu[:])
            bmus = cpool.tile([Z, 1], F32, tag="bmus")
            nc.sync.dma_start(bmus[:], bmu[:])
            Wlvs = cpool.tile([H, Z], F32, tag="Wlvs")
            nc.sync.dma_start(Wlvs[:], Wlv[:])
            blvs = cpool.tile([Z, 1], F32, tag="blvs")
            nc.sync.dma_start(blvs[:], blv[:])
            epsTs = cpool.tile([Z, R], F32, tag="epsTs")
            nc.sync.dma_start(epsTs[:], epsT[:])

            # DRAM bounce buffers for collectives
            dinv_in = dram.tile([1, R], F32, tag="dinv_in")
            dinv_out = dram.tile([NCORES, R], F32, tag="dinv_out")
            hs_in = dram.tile([P, RT * H], BF16, tag="hs_in")
            hs_out = dram.tile([NCORES * P, RT * H], BF16, tag="hs_out")
            z_in = dram.tile([Z, R], F32, tag="z_in")
            z_out = dram.tile([NCORES * Z, R], F32, tag="z_out")

            # persistent activations
            hsT = ppool.tile([H, R], F32, tag="hsT")          # (S_r h_r)^T
            bc = ppool.tile([P, R], F32, tag="bc")            # dinv_r bcast to rows
            xs_all = ppool.tile([P, KT * F], BF16, tag="xs_all")
            xs_own = ppool.tile([P, RT * F], F32, tag="xs_own")
            hs_all = ppool.tile([P, KT * H], BF16, tag="hs_all")
            zT_loc = ppool.tile([Z, R], F32, tag="zT_loc")
            zT_full = ppool.tile([Z, N], F32, tag="zT_full")
            dinv_pt = ppool.tile([P, KT], F32, tag="dinv_pt")
            dinv_own = ppool.tile([P, RT], F32, tag="dinv_own")
            dinv_row = ppool.tile([1, R], F32, tag="dinv_row")

            # ================= phase B: degree =================
            with (
                tc.tile_pool(name="stream_deg", bufs=stream_bufs) as spool,
                tc.tile_pool(name="psum_deg", bufs=1, space="PSUM") as psdeg,
                tc.tile_pool(name="tmp_deg", bufs=1) as tpool,
            ):
                deg_ps = psdeg.tile([1, R], F32, tag="deg_ps")
                for kk in range(KT):
                    at = spool.tile([P, R], BF16, tag="at")
                    nc.sync.dma_start(at[:], aT[kk * P:(kk + 1) * P, :])
                    for c in range(R // 512):
                        nc.tensor.matmul(
                            deg_ps[:, c * 512:(c + 1) * 512],
                            ones_b[:],
                            at[:, c * 512:(c + 1) * 512],
                            start=(kk == 0),
                            stop=(kk == KT - 1),
                        )
                deg_sb = tpool.tile([1, R], F32, tag="deg_sb")
                nc.vector.tensor_scalar_add(deg_sb[:], deg_ps[:], 1.0)
                rcp_sb = tpool.tile([1, R], F32, tag="rcp_sb")
                nc.vector.reciprocal(rcp_sb[:], deg_sb[:])
                nc.scalar.activation(
                    dinv_row[:], rcp_sb[:], mybir.ActivationFunctionType.Sqrt
                )
                nc.sync.dma_start(dinv_in[:], dinv_row[:])
                nc.gpsimd.collective_compute(
                    "AllGather",
                    mybir.AluOpType.bypass,
                    replica_groups=RG,
                    ins=[dinv_in[:].opt()],
                    outs=[dinv_out[:].opt()],
                )

            # ============ phase C: dinv layouts + scaled x ============
            with (
                tc.tile_pool(name="psum_c", bufs=1, space="PSUM") as psc,
                tc.tile_pool(name="tmp_c", bufs=1) as tpool,
            ):
                dl = tpool.tile([KT, P], F32, tag="dl")
                nc.sync.dma_start(
                    dl[:],
                    dinv_out[:].rearrange("a b -> (a b)").rearrange(
                        "(k p) -> k p", p=P
                    ),
                )
                dl_ps = psc.tile([P, KT], F32, tag="dl_ps")
                nc.tensor.matmul(
                    dl_ps[:], dl[:], identf[:KT, :KT], is_transpose=True,
                    start=True, stop=True,
                )
                nc.vector.tensor_copy(dinv_pt[:], dl_ps[:])

                do = tpool.tile([RT, P], F32, tag="do")
                nc.sync.dma_start(
                    do[:],
                    dinv_in[:].rearrange("a b -> (a b)").rearrange(
                        "(k p) -> k p", p=P
                    ),
                )
                do_ps = psc.tile([P, RT], F32, tag="do_ps")
                nc.tensor.matmul(
                    do_ps[:], do[:], identf[:RT, :RT], is_transpose=True,
                    start=True, stop=True,
                )
                nc.vector.tensor_copy(dinv_own[:], do_ps[:])

                bc_ps = psc.tile([P, R], F32, tag="bc_ps")
                for c in range(R // 512):
                    nc.tensor.matmul(
                        bc_ps[:, c * 512:(c + 1) * 512],
                        ones_row[:],
                        dinv_row[:, c * 512:(c + 1) * 512],
                        start=True, stop=True,
                    )
                nc.vector.tensor_copy(bc[:], bc_ps[:])

                xps = tpool.tile([P, KT * F], F32, tag="xps")
                nc.sync.dma_start(xps[:], xp[:])
                xops = tpool.tile([P, RT * F], F32, tag="xops")
                nc.sync.dma_start(xops[:], xop[:])
                for kk in range(KT):
                    nc.vector.tensor_scalar_mul(
                        xs_all[:, kk * F:(kk + 1) * F],
                        xps[:, kk * F:(kk + 1) * F],
                        dinv_pt[:, kk:kk + 1],
                    )
                for t in range(RT):
                    nc.vector.tensor_scalar_mul(
                        xs_own[:, t * F:(t + 1) * F],
                        xops[:, t * F:(t + 1) * F],
                        dinv_own[:, t:t + 1],
                    )

            # ================= phase D: conv1 =================
            with (
                tc.tile_pool(name="stream_p1", bufs=stream_bufs) as spool,
                tc.tile_pool(name="psum_p1", bufs=1, space="PSUM") as ps1,
                tc.tile_pool(name="psum_tr", bufs=2, space="PSUM") as pstr,
                tc.tile_pool(name="tmp_d", bufs=1) as tpool,
            ):
                out1 = ps1.tile([F, R], F32, tag="out1")
                for kk in range(KT):
                    at = spool.tile([P, R], BF16, tag="at")
                    nc.sync.dma_start(at[:], aT[kk * P:(kk + 1) * P, :])
                    for c in range(R // 512):
                        nc.tensor.matmul(
                            out1[:, c * 512:(c + 1) * 512],
                            xs_all[:, kk * F:(kk + 1) * F],
                            at[:, c * 512:(c + 1) * 512],
                            start=(kk == 0),
                            stop=False,
                        )
                # self-loop: out1[:, t*128:(t+1)*128] += xs_own_tile^T
                last_bank = {}
                for t in range(RT):
                    last_bank[(t * P) // 512] = t
                for t in range(RT):
                    nc.tensor.matmul(
                        out1[:, t * P:(t + 1) * P],
                        xs_own[:, t * F:(t + 1) * F],
                        identf[:],
                        is_transpose=True,
                        start=False,
                        stop=(last_bank[(t * P) // 512] == t),
                    )
                t1u = tpool.tile([F, R], F32, tag="t1u")
                nc.vector.tensor_copy(t1u[:], out1[:])
                w1o = ps1.tile([H, R], F32, tag="w1o")
                for c in range(R // 512):
                    nc.tensor.matmul(
                        w1o[:, c * 512:(c + 1) * 512],
                        W1s[:],
                        t1u[:, c * 512:(c + 1) * 512],
                        start=True, stop=True,
                    )
                t1s = tpool.tile([H, R], F32, tag="t1s")
                nc.vector.tensor_mul(t1s[:], w1o[:], bc[:])
                hT = tpool.tile([H, R], F32, tag="hT")
                nc.scalar.activation(
                    hT[:], t1s[:], mybir.ActivationFunctionType.Relu, bias=b1s[:]
                )
                nc.vector.tensor_mul(hsT[:], hT[:], bc[:])
                hs_pack = tpool.tile([P, RT * H], BF16, tag="hs_pack")
                for t in range(RT):
                    tr_ps = pstr.tile([P, P], F32, tag="tr_ps")
                    nc.tensor.matmul(
                        tr_ps[:],
                        hsT[:, t * P:(t + 1) * P],
                        identf[:],
                        is_transpose=True,
                        start=True, stop=True,
                    )
                    nc.vector.tensor_copy(hs_pack[:, t * H:(t + 1) * H], tr_ps[:])
                nc.sync.dma_start(hs_in[:], hs_pack[:])
                nc.gpsimd.collective_compute(
                    "AllGather",
                    mybir.AluOpType.bypass,
                    replica_groups=RG,
                    ins=[hs_in[:].opt()],
                    outs=[hs_out[:].opt()],
                )
                for r in range(NCORES):
                    nc.sync.dma_start(
                        hs_all[:, r * RT * H:(r + 1) * RT * H],
                        hs_out[r * P:(r + 1) * P, :],
                    )

            # ================= phase E: conv2 + heads =================
            with (
                tc.tile_pool(name="stream_p2", bufs=stream_bufs) as spool,
                tc.tile_pool(name="psum_p2", bufs=1, space="PSUM") as ps2,
                tc.tile_pool(name="tmp_e", bufs=1) as tpool,
            ):
                out2 = ps2.tile([H, R], F32, tag="out2")
                for kk in range(KT):
                    at = spool.tile([P, R], BF16, tag="at")
                    nc.sync.dma_start(at[:], aT[kk * P:(kk + 1) * P, :])
                    for c in range(R // 512):
                        nc.tensor.matmul(
                            out2[:, c * 512:(c + 1) * 512],
                            hs_all[:, kk * H:(kk + 1) * H],
                            at[:, c * 512:(c + 1) * 512],
                            start=(kk == 0),
                            stop=(kk == KT - 1),
                        )
                ahu = tpool.tile([H, R], F32, tag="ahu")
                nc.vector.tensor_add(ahu[:], out2[:], hsT[:])
                mu_ps = ps2.tile([Z, R], F32, tag="mu_ps")
                lv_ps = ps2.tile([Z, R], F32, tag="lv_ps")
                for c in range(R // 512):
                    nc.tensor.matmul(
                        mu_ps[:, c * 512:(c + 1) * 512], Wmus[:],
                        ahu[:, c * 512:(c + 1) * 512], start=True, stop=True,
                    )
                    nc.tensor.matmul(
                        lv_ps[:, c * 512:(c + 1) * 512], Wlvs[:],
                        ahu[:, c * 512:(c + 1) * 512], start=True, stop=True,
                    )
                muT = tpool.tile([Z, R], F32, tag="muT")
                nc.vector.tensor_mul(muT[:], mu_ps[:], bc[:Z, :])
                nc.vector.tensor_scalar_add(muT[:], muT[:], bmus[:])
                lvT = tpool.tile([Z, R], F32, tag="lvT")
                nc.vector.tensor_mul(lvT[:], lv_ps[:], bc[:Z, :])
                nc.vector.tensor_scalar_add(lvT[:], lvT[:], blvs[:])
                nc.sync.dma_start(muT_o[:], muT[:])
                nc.sync.dma_start(lvT_o[:], lvT[:])
                esd = tpool.tile([Z, R], F32, tag="esd")
                nc.scalar.activation(
                    esd[:], lvT[:], mybir.ActivationFunctionType.Exp, scale=0.5
                )
                prod = tpool.tile([Z, R], F32, tag="prod")
                nc.vector.tensor_mul(prod[:], epsTs[:], esd[:])
                nc.vector.tensor_add(zT_loc[:], muT[:], prod[:])
                nc.sync.dma_start(z_in[:], zT_loc[:])
                nc.gpsimd.collective_compute(
                    "AllGather",
                    mybir.AluOpType.bypass,
                    replica_groups=RG,
                    ins=[z_in[:].opt()],
                    outs=[z_out[:].opt()],
                )
                for r in range(NCORES):
                    nc.sync.dma_start(
                        zT_full[:, r * R:(r + 1) * R],
                        z_out[r * Z:(r + 1) * Z, :],
                    )

            # ================= phase F: decode =================
            with (
                tc.tile_pool(name="rowbuf", bufs=2) as rpool,
                tc.tile_pool(name="psum_dec", bufs=4, space="PSUM") as psd,
            ):
                for mt in range(RT):
                    row = rpool.tile([P, N], F32, tag="row")
                    for ch in range(NCH):
                        dps = psd.tile([P, 512], F32, tag="dps")
                        nc.tensor.matmul(
                            dps[:],
                            zT_loc[:, mt * P:(mt + 1) * P],
                            zT_full[:, ch * 512:(ch + 1) * 512],
                            start=True, stop=True,
                        )
                        nc.scalar.activation(
                            row[:, ch * 512:(ch + 1) * 512], dps[:],
                            mybir.ActivationFunctionType.Sigmoid,
                        )
                    nc.sync.dma_start(rec[mt * P:(mt + 1) * P, :], row[:])
    nc.compile()
    return nc


def _prep_inputs(x, adj, eps, W1, b1, Wmu, bmu, Wlv, blv, N):
    import ml_dtypes

    R = N // NCORES
    KT = N // P
    RT = R // P
    x = np.asarray(x, np.float32)
    adj = np.asarray(adj, np.float32)
    eps = np.asarray(eps, np.float32)
    xp = np.ascontiguousarray(
        x.reshape(KT, P, F).transpose(1, 0, 2).reshape(P, KT * F)
    )
    in_maps = []
    for c in range(NCORES):
        rows = slice(c * R, (c + 1) * R)
        aT = np.ascontiguousarray(adj[rows, :].T).astype(ml_dtypes.bfloat16)
        xo = x[rows].reshape(RT, P, F).transpose(1, 0, 2).reshape(P, RT * F)
        in_maps.append(
            {
                "aT": aT,
                "xp": xp,
                "xop": np.ascontiguousarray(xo),
                "epsT": np.ascontiguousarray(eps[rows].T),
                "W1": np.asarray(W1, np.float32),
                "b1": np.asarray(b1, np.float32).reshape(H, 1),
                "Wmu": np.asarray(Wmu, np.float32),
                "bmu": np.asarray(bmu, np.float32).reshape(Z, 1),
                "Wlv": np.asarray(Wlv, np.float32),
                "blv": np.asarray(blv, np.float32).reshape(Z, 1),
            }
        )
    return in_maps


def _assemble(results):
    rec = np.concatenate([r["rec"] for r in results], axis=0)
    mu = np.concatenate([r["muT"].T for r in results], axis=0)
    lv = np.concatenate([r["lvT"].T for r in results], axis=0)
    return rec, np.ascontiguousarray(mu), np.ascontiguousarray(lv)


def kernel(x, adj, eps, W1, b1, Wmu, bmu, Wlv, blv, _trace=False):
    N = np.asarray(adj).shape[0]
    key = (N,)
    if key not in _CACHE:
        _CACHE[key] = build_bass(N)
    nc = _CACHE[key]
    in_maps = _prep_inputs(x, adj, eps, W1, b1, Wmu, bmu, Wlv, blv, N)
    res = bass_utils.run_bass_kernel_spmd(
        nc, in_maps, core_ids=list(range(NCORES)), trace=_trace
    )
    _CACHE["last_result"] = res
    return _assemble(res.results)
